# revision 1
# baseline (speedup 1.0000x reference)
"""Trainium2 Bass kernel for nn_ChannelSelfAttention.

Reference computation (per batch sample b):
    xt   = x[b].T                          # [C, L]
    q    = xt @ Wq.T + bq                  # [C, H]
    kv   = xt @ Wkv.T + bkv                # [C, 2H] -> k, v
    attn = (q * H**-0.5) @ k.T             # [C, C]  (no softmax)
    y    = attn @ v                        # [C, H]
    g    = mean(y, axis=-1)                # [C]
    out[b] = x[b] * g[None, :]             # [L, C]

Sharding: data-parallel over B across 8 cores (4 samples per core);
weights replicated. Each sample's x (4 MiB) is held fully in SBUF, so
HBM traffic per core is read 16 MiB + write 16 MiB + 3 MiB weights —
the memory roofline for this problem.

On-device layout notes (per sample):
  - x in SBUF as [p=128, n=32, c=256], p+128n = l (the L dim).
  - qkv^T computed by accumulating W_all @ x[b] over the 32 l-chunks
    (lhsT = W_all^T chunk, rhs = x chunk), giving q^T [64, 256] and
    kv^T [128, 256] in PSUM with the contraction over partitions.
  - attn^T[d, c] computed directly (lhsT = k^T d-chunk, rhs = q^T) to
    avoid transposing attn for the y matmul.
  - v^T [64, 256] is PE-transposed to v [256, 64] (two 64x128 tiles) so
    y^T = sum_d v[d, h] attn^T[d, c] accumulates naturally.
  - mean over H folded into a broadcast matmul: lhsT = (1/H) ones
    [64, 128], rhs = y^T -> g broadcast to all 128 partitions in one go.
  - gate: one DVE tensor_tensor multiply over the whole sample with g
    broadcast along the n axis via a stride-0 access pattern.
"""

import numpy as np

import concourse.bass as bass
import concourse.mybir as mybir
import concourse.tile as tile
from concourse import bacc
from concourse.bass_utils import run_bass_kernel_spmd

B, L, C, H = 32, 4096, 256, 64
N_CORES = 8
B_LOC = B // N_CORES          # samples per core
P = 128                       # SBUF partitions
JC = 4                        # L-rows per partition per chunk (4KB DMA descs)
NCH = L // (P * JC)           # l-chunks per sample (8)
DCH = C // P                  # d-chunks (2)
F32 = mybir.dt.float32
F32R = mybir.dt.float32r
SCALE = float(H) ** -0.5


def _r(ap):
    """Bitcast an f32 AP to float32r: PE runs 1 cycle/row (vs 4 for f32)
    when the output free dim is >= 256."""
    return ap.bitcast(F32R)


def _emit(
    tc: "tile.TileContext", x_d, wT_d, bq_d, bkv_d, id_d, ones_d, out_d
) -> None:
    nc = tc.nc
    with (
        tc.tile_pool(name="singles", bufs=1) as singles,
        tc.tile_pool(name="xio", bufs=2) as xio,
        tc.tile_pool(name="small", bufs=2) as small,
        tc.tile_pool(name="psum2", bufs=2, space="PSUM") as psum2,
        tc.tile_pool(name="psum1", bufs=1, space="PSUM") as psum1,
    ):
        # ---- one-time loads / constants (scalar HWDGE ring, so they
        # overlap the first x load on the sync ring) ----
        # W_all^T as [p, n, j, 3H]: 3KB contiguous DRAM per (p, n) descriptor.
        # First chunk loaded separately so sample 0's first matmuls gate on
        # 384KB of weights, not 3MB.
        wT_sb = singles.tile([P, NCH, JC, 3 * H], F32R)      # 3 MiB
        wT_src = wT_d[:].rearrange("(n p j) h -> p n j h", p=P, j=JC)
        nc.scalar.dma_start(out=wT_sb[:, 0:1], in_=wT_src[:, 0:1])
        nc.scalar.dma_start(out=wT_sb[:, 1:NCH], in_=wT_src[:, 1:NCH])
        bq_sb = singles.tile([H, 1], F32)
        nc.scalar.dma_start(out=bq_sb, in_=bq_d[:].rearrange("(h o) -> h o", o=1))
        bkv_sb = singles.tile([2 * H, 1], F32)
        nc.scalar.dma_start(
            out=bkv_sb, in_=bkv_d[:].rearrange("(h o) -> h o", o=1)
        )
        # 64x64 identity living at partitions 64:128 so the v^T transpose
        # (lhsT at base partition 64) has a base-aligned rhs.
        id_tile = singles.tile([P, H], F32R)
        nc.scalar.dma_start(out=id_tile[H:P, :], in_=id_d[:])
        ident_hi = id_tile[H:P, :]
        ones_h = singles.tile([H, P], F32R)                  # filled with 1/H
        nc.scalar.dma_start(out=ones_h, in_=ones_d[:])

        HALF = NCH // 2
        for b in range(B_LOC):
            # ---- load x[b] into SBUF: [128, 8, 4*256] ----
            # l = n*512 + p*4 + j, so each (p, n) descriptor moves 4KB of
            # contiguous DRAM. Two half-loads for pipelining.
            x_sb = xio.tile([P, NCH, JC * C], F32R, tag="x")
            x_src = x_d[b].rearrange("(n p j) c -> p n (j c)", p=P, j=JC)
            for hh in range(2):
                sl = slice(hh * HALF, (hh + 1) * HALF)
                nc.sync.dma_start(out=x_sb[:, sl, :], in_=x_src[:, sl, :])

            # ---- qkv^T = W_all @ x[b]: accumulate over 8 chunks x 4 j ----
            psum_q = psum2.tile([H, C], F32, tag="q")
            psum_kv = psum2.tile([2 * H, C], F32, tag="kv")
            for n in range(NCH):
                for j in range(JC):
                    nc.tensor.matmul(
                        psum_q,
                        lhsT=wT_sb[:, n, j, 0:H],
                        rhs=x_sb[:, n, j * C : (j + 1) * C],
                        start=(n == 0 and j == 0),
                        stop=(n == NCH - 1 and j == JC - 1),
                    )
            for n in range(NCH):
                for j in range(JC):
                    nc.tensor.matmul(
                        psum_kv,
                        lhsT=wT_sb[:, n, j, H : 3 * H],
                        rhs=x_sb[:, n, j * C : (j + 1) * C],
                        start=(n == 0 and j == 0),
                        stop=(n == NCH - 1 and j == JC - 1),
                    )

            # q^T scaled+biased; kv^T biased (per-partition bias)
            q_sb = small.tile([H, C], F32R, tag="q_sb")
            nc.vector.tensor_scalar(
                out=q_sb,
                in0=psum_q,
                scalar1=bq_sb,
                scalar2=SCALE,
                op0=mybir.AluOpType.add,
                op1=mybir.AluOpType.mult,
            )
            kv_sb = small.tile([2 * H, C], F32R, tag="kv_sb")
            nc.vector.tensor_scalar(
                out=kv_sb,
                in0=psum_kv,
                scalar1=bkv_sb,
                scalar2=None,
                op0=mybir.AluOpType.add,
            )
            kT = kv_sb[0:H, :]                    # [64, 256]
            vT = kv_sb[H : 2 * H, :]              # [64, 256]

            # ---- v natural [d, h]: PE-transpose the two vT halves ----
            psum_vt = psum1.tile([P, P], F32R, tag="vt")
            for d in range(DCH):
                nc.tensor.transpose(
                    psum_vt[:, d * H : (d + 1) * H],
                    vT[:, d * P : (d + 1) * P],
                    ident_hi,
                )
            v_sb = small.tile([P, P], F32R, tag="v_sb")
            nc.scalar.copy(v_sb, psum_vt)

            # ---- attn^T[d, c] = sum_h k^T[h, d] * q^T[h, c] ----
            psum_at = psum1.tile([P, DCH * C], F32, tag="at")
            for d in range(DCH):
                nc.tensor.matmul(
                    psum_at[:, d * C : (d + 1) * C],
                    lhsT=kT[:, d * P : (d + 1) * P],
                    rhs=q_sb[:],
                )
            at_sb = small.tile([P, DCH * C], F32R, tag="at_sb")
            nc.scalar.copy(at_sb, psum_at)

            # ---- y^T[h, c] = sum_d v[d, h] * attn^T[d, c] ----
            psum_yt = psum1.tile([H, C], F32, tag="yt")
            for d in range(DCH):
                nc.tensor.matmul(
                    psum_yt,
                    lhsT=v_sb[:, d * H : (d + 1) * H],
                    rhs=at_sb[:, d * C : (d + 1) * C],
                    start=(d == 0),
                    stop=(d == DCH - 1),
                )
            yt_sb = small.tile([H, C], F32R, tag="yt_sb")
            nc.scalar.copy(yt_sb, psum_yt)

            # ---- g = mean_h y^T, broadcast to all 128 partitions ----
            psum_g = psum1.tile([P, C], F32, tag="g")
            nc.tensor.matmul(psum_g, lhsT=ones_h[:], rhs=yt_sb[:])
            g_sb = small.tile([P, C], F32, tag="g_sb")
            nc.scalar.copy(g_sb, psum_g)

            # ---- gate: out = x * g (g broadcast along n,j via stride 0),
            # split in halves so each half's store overlaps the other ----
            out_sb = xio.tile([P, NCH, JC * C], F32, tag="out")
            out_dst = out_d[b].rearrange("(n p j) c -> p n (j c)", p=P, j=JC)
            g_bc = bass.AP(
                tensor=g_sb.tensor,
                offset=g_sb.offset,
                ap=[list(g_sb.ap[0]), [0, HALF], [0, JC], list(g_sb.ap[1])],
            )
            for hh in range(2):
                sl = slice(hh * HALF, (hh + 1) * HALF)
                nc.vector.tensor_tensor(
                    out=out_sb[:, sl, :].rearrange(
                        "p n (j c) -> p n j c", j=JC
                    ),
                    in0=x_sb[:, sl, :]
                    .bitcast(F32)
                    .rearrange("p n (j c) -> p n j c", j=JC),
                    in1=g_bc,
                    op=mybir.AluOpType.mult,
                )
                nc.scalar.dma_start(out=out_dst[:, sl, :], in_=out_sb[:, sl, :])


def build():
    nc = bacc.Bacc(
        "TRN2", target_bir_lowering=False, debug=False, num_devices=N_CORES
    )
    x_d = nc.dram_tensor("x", [B_LOC, L, C], F32R, kind="ExternalInput")
    wT_d = nc.dram_tensor("wT", [L, 3 * H], F32R, kind="ExternalInput")
    bq_d = nc.dram_tensor("bq", [H], F32, kind="ExternalInput")
    bkv_d = nc.dram_tensor("bkv", [2 * H], F32, kind="ExternalInput")
    id_d = nc.dram_tensor("ident", [H, H], F32R, kind="ExternalInput")
    ones_d = nc.dram_tensor("ones", [H, P], F32R, kind="ExternalInput")
    out_d = nc.dram_tensor("out", [B_LOC, L, C], F32, kind="ExternalOutput")
    with tile.TileContext(nc) as tc:
        _emit(tc, x_d, wT_d, bq_d, bkv_d, id_d, ones_d, out_d)
    nc.compile()
    return nc


_nc_cache = None


def _get_nc():
    global _nc_cache
    if _nc_cache is None:
        _nc_cache = build()
    return _nc_cache


def make_in_maps(x, Wq, bq, Wkv, bkv):
    x = np.ascontiguousarray(np.asarray(x, dtype=np.float32))
    wT = np.ascontiguousarray(
        np.concatenate(
            [np.asarray(Wq, np.float32), np.asarray(Wkv, np.float32)], axis=0
        ).T
    )
    bq = np.ascontiguousarray(np.asarray(bq, np.float32))
    bkv = np.ascontiguousarray(np.asarray(bkv, np.float32))
    ident = np.eye(H, dtype=np.float32)
    ones = np.full((H, P), 1.0 / H, dtype=np.float32)
    return [
        {
            "x": np.ascontiguousarray(x[i * B_LOC : (i + 1) * B_LOC]),
            "wT": wT,
            "bq": bq,
            "bkv": bkv,
            "ident": ident,
            "ones": ones,
        }
        for i in range(N_CORES)
    ]


def run(inputs, **spmd_kwargs):
    """Run on hardware; returns (full_output, BassKernelResults)."""
    nc = _get_nc()
    in_maps = make_in_maps(**inputs)
    res = run_bass_kernel_spmd(nc, in_maps, list(range(N_CORES)), **spmd_kwargs)
    out = np.concatenate([r["out"] for r in res.results], axis=0)
    return out, res


def kernel(**inputs) -> np.ndarray:
    out, _ = run(inputs)
    return out



# revision 4
# speedup vs baseline: 1.0126x; 1.0126x over previous
"""Trainium2 Bass kernel for nn_ChannelSelfAttention.

Reference computation (per batch sample b):
    xt   = x[b].T                          # [C, L]
    q    = xt @ Wq.T + bq                  # [C, H]
    kv   = xt @ Wkv.T + bkv                # [C, 2H] -> k, v
    attn = (q * H**-0.5) @ k.T             # [C, C]  (no softmax)
    y    = attn @ v                        # [C, H]
    g    = mean(y, axis=-1)                # [C]
    out[b] = x[b] * g[None, :]             # [L, C]

Sharding: data-parallel over B across 8 cores (4 samples per core);
weights replicated. Per-core HBM traffic: 16 MiB x read + 16 MiB out
write + 3 MiB weights = 36.75 MB -> ~103 us at the 358 GB/s per-core
HBM roofline.

Pipeline design (v2):
  - x tiles come from a bufs=4 pool, one buffer per sample, so all four
    x loads issue back-to-back on the sync ring with no buffer-reuse
    waits (the v1 bufs=2 layout head-of-line blocked the sync ring for
    ~24 us per sample waiting on the gate DVE op to free a buffer).
  - The gate multiply runs IN PLACE over the x tile (out = x * g), so
    no separate output tile is needed; the store DMAs straight from the
    x tile. Gate+store run at quarter-sample granularity to shorten the
    critical tail after the last x load.
  - l = p*32 + n*4 + j (partition outermost), so each partition's slice
    of x / out is one contiguous 32 KiB run of DRAM (16 KiB per
    half-load descriptor, 8 KiB per quarter-store descriptor).
  - PSUM->SBUF copies run on DVE (vector), not ACT, so the ACT ring
    only carries weight loads + output stores and a store blocked on a
    gate cannot delay the attention-chain copies.

On-device layout notes (per sample):
  - x in SBUF as [p=128, n=8, (j=4)*(c=256)].
  - qkv^T computed by accumulating W_all @ x[b] over the 32 (n,j)
    chunks (lhsT = W_all^T chunk, rhs = x chunk), giving q^T [64, 256]
    and kv^T [128, 256] in PSUM with the contraction over partitions.
  - attn^T[d, c] computed directly (lhsT = k^T d-chunk, rhs = q^T) to
    avoid transposing attn for the y matmul.
  - v^T [64, 256] is PE-transposed to v [256, 64] (two 64x128 tiles) so
    y^T = sum_d v[d, h] attn^T[d, c] accumulates naturally.
  - mean over H folded into a broadcast matmul: lhsT = (1/H) ones
    [64, 128], rhs = y^T -> g broadcast to all 128 partitions.
  - gate: DVE tensor_tensor multiply, g broadcast along the n,j axes
    via a stride-0 access pattern, written back over the x tile.
"""

import numpy as np

import concourse.bass as bass
import concourse.mybir as mybir
import concourse.tile as tile
from concourse import bacc
from concourse.bass_utils import run_bass_kernel_spmd

B, L, C, H = 32, 4096, 256, 64
N_CORES = 8
B_LOC = B // N_CORES          # samples per core
P = 128                       # SBUF partitions
JC = 4                        # L-rows per partition per chunk
NCH = L // (P * JC)           # l-chunks per sample (8)
DCH = C // P                  # d-chunks (2)
NQ = NCH // 4                 # n-chunks per gate/store quarter (2)
F32 = mybir.dt.float32
F32R = mybir.dt.float32r
SCALE = float(H) ** -0.5


def _emit(
    tc: "tile.TileContext", x_d, wT_d, bq_d, bkv_d, id_d, ones_d, out_d
) -> None:
    nc = tc.nc
    with (
        tc.tile_pool(name="singles", bufs=1) as singles,
        tc.tile_pool(name="xio", bufs=3) as xio,
        tc.tile_pool(name="oq", bufs=3) as oqp,
        tc.tile_pool(name="small", bufs=2) as small,
        tc.tile_pool(name="psum2", bufs=2, space="PSUM") as psum2,
        tc.tile_pool(name="psum1", bufs=1, space="PSUM") as psum1,
    ):
        # ---- one-time loads / constants (scalar HWDGE ring, so they
        # overlap the x loads on the sync ring) ----
        # W_all^T as [p, n, j, 3H]: l = p*32 + n*4 + j, so each
        # partition's weight slice is 24 KiB of contiguous DRAM.
        # First chunk loaded separately so sample 0's first matmuls gate
        # on 384KB of weights, not 3MB.
        wT_sb = singles.tile([P, NCH, JC, 3 * H], F32R)      # 3 MiB
        wT_src = wT_d[:].rearrange("(p n j) h -> p n j h", n=NCH, j=JC)
        nc.scalar.dma_start(out=wT_sb[:, 0:1], in_=wT_src[:, 0:1])
        nc.scalar.dma_start(out=wT_sb[:, 1:NCH], in_=wT_src[:, 1:NCH])
        bq_sb = singles.tile([H, 1], F32)
        nc.scalar.dma_start(out=bq_sb, in_=bq_d[:].rearrange("(h o) -> h o", o=1))
        bkv_sb = singles.tile([2 * H, 1], F32)
        nc.scalar.dma_start(
            out=bkv_sb, in_=bkv_d[:].rearrange("(h o) -> h o", o=1)
        )
        # 64x64 identity living at partitions 64:128 so the v^T transpose
        # (lhsT at base partition 64) has a base-aligned rhs.
        id_tile = singles.tile([P, H], F32R)
        nc.scalar.dma_start(out=id_tile[H:P, :], in_=id_d[:])
        ident_hi = id_tile[H:P, :]
        ones_h = singles.tile([H, P], F32R)                  # filled with 1/H
        nc.scalar.dma_start(out=ones_h, in_=ones_d[:])

        HALF = NCH // 2
        for b in range(B_LOC):
            # ---- load x[b] into SBUF: [128, 8, 4*256] ----
            # l = p*32 + n*4 + j: per (partition, half) the DRAM source
            # is one 16 KiB contiguous run. Two half-loads so the qkv
            # matmuls start on half 0 while half 1 is in flight.
            x_sb = xio.tile([P, NCH, JC * C], F32R, tag="x")
            x_src = x_d[b].rearrange("(p n j) c -> p n (j c)", n=NCH, j=JC)
            for hh in range(2):
                sl = slice(hh * HALF, (hh + 1) * HALF)
                nc.sync.dma_start(out=x_sb[:, sl, :], in_=x_src[:, sl, :])

            # ---- qkv^T = W_all @ x[b]: accumulate over 8 chunks x 4 j ----
            psum_q = psum2.tile([H, C], F32, tag="q")
            psum_kv = psum2.tile([2 * H, C], F32, tag="kv")
            for n in range(NCH):
                for j in range(JC):
                    nc.tensor.matmul(
                        psum_q,
                        lhsT=wT_sb[:, n, j, 0:H],
                        rhs=x_sb[:, n, j * C : (j + 1) * C],
                        start=(n == 0 and j == 0),
                        stop=(n == NCH - 1 and j == JC - 1),
                    )
            for n in range(NCH):
                for j in range(JC):
                    nc.tensor.matmul(
                        psum_kv,
                        lhsT=wT_sb[:, n, j, H : 3 * H],
                        rhs=x_sb[:, n, j * C : (j + 1) * C],
                        start=(n == 0 and j == 0),
                        stop=(n == NCH - 1 and j == JC - 1),
                    )

            # q^T scaled+biased; kv^T biased (per-partition bias)
            q_sb = small.tile([H, C], F32R, tag="q_sb")
            nc.vector.tensor_scalar(
                out=q_sb,
                in0=psum_q,
                scalar1=bq_sb,
                scalar2=SCALE,
                op0=mybir.AluOpType.add,
                op1=mybir.AluOpType.mult,
            )
            kv_sb = small.tile([2 * H, C], F32R, tag="kv_sb")
            nc.vector.tensor_scalar(
                out=kv_sb,
                in0=psum_kv,
                scalar1=bkv_sb,
                scalar2=None,
                op0=mybir.AluOpType.add,
            )
            kT = kv_sb[0:H, :]                    # [64, 256]
            vT = kv_sb[H : 2 * H, :]              # [64, 256]

            # ---- v natural [d, h]: PE-transpose the two vT halves ----
            psum_vt = psum1.tile([P, P], F32R, tag="vt")
            for d in range(DCH):
                nc.tensor.transpose(
                    psum_vt[:, d * H : (d + 1) * H],
                    vT[:, d * P : (d + 1) * P],
                    ident_hi,
                )
            v_sb = small.tile([P, P], F32R, tag="v_sb")
            nc.vector.tensor_copy(v_sb, psum_vt)

            # ---- attn^T[d, c] = sum_h k^T[h, d] * q^T[h, c] ----
            psum_at = psum1.tile([P, DCH * C], F32, tag="at")
            for d in range(DCH):
                nc.tensor.matmul(
                    psum_at[:, d * C : (d + 1) * C],
                    lhsT=kT[:, d * P : (d + 1) * P],
                    rhs=q_sb[:],
                )
            at_sb = small.tile([P, DCH * C], F32R, tag="at_sb")
            nc.vector.tensor_copy(at_sb, psum_at)

            # ---- y^T[h, c] = sum_d v[d, h] * attn^T[d, c] ----
            psum_yt = psum1.tile([H, C], F32, tag="yt")
            for d in range(DCH):
                nc.tensor.matmul(
                    psum_yt,
                    lhsT=v_sb[:, d * H : (d + 1) * H],
                    rhs=at_sb[:, d * C : (d + 1) * C],
                    start=(d == 0),
                    stop=(d == DCH - 1),
                )
            yt_sb = small.tile([H, C], F32R, tag="yt_sb")
            nc.vector.tensor_copy(yt_sb, psum_yt)

            # ---- g = mean_h y^T, broadcast to all 128 partitions ----
            psum_g = psum1.tile([P, C], F32, tag="g")
            nc.tensor.matmul(psum_g, lhsT=ones_h[:], rhs=yt_sb[:])
            g_sb = small.tile([P, C], F32, tag="g_sb")
            nc.vector.tensor_copy(g_sb, psum_g)

            # ---- gate (out = x * g, g broadcast along n,j via stride 0)
            # and store, at quarter granularity so each store overlaps
            # the next quarter's gate. The staging tile is separate from
            # x_sb (an in-place gate trips the FP32r matmul verifier). ----
            out_dst = out_d[b].rearrange("(p n j) c -> p n (j c)", n=NCH, j=JC)
            g_bc = bass.AP(
                tensor=g_sb.tensor,
                offset=g_sb.offset,
                ap=[list(g_sb.ap[0]), [0, NQ], [0, JC], list(g_sb.ap[1])],
            )
            for qq in range(4):
                sl = slice(qq * NQ, (qq + 1) * NQ)
                oq = oqp.tile([P, NQ, JC * C], F32, tag="oq")
                nc.vector.tensor_tensor(
                    out=oq.rearrange("p n (j c) -> p n j c", j=JC),
                    in0=x_sb[:, sl, :]
                    .bitcast(F32)
                    .rearrange("p n (j c) -> p n j c", j=JC),
                    in1=g_bc,
                    op=mybir.AluOpType.mult,
                )
                nc.scalar.dma_start(out=out_dst[:, sl, :], in_=oq)


def build():
    nc = bacc.Bacc(
        "TRN2", target_bir_lowering=False, debug=False, num_devices=N_CORES
    )
    x_d = nc.dram_tensor("x", [B_LOC, L, C], F32R, kind="ExternalInput")
    wT_d = nc.dram_tensor("wT", [L, 3 * H], F32R, kind="ExternalInput")
    bq_d = nc.dram_tensor("bq", [H], F32, kind="ExternalInput")
    bkv_d = nc.dram_tensor("bkv", [2 * H], F32, kind="ExternalInput")
    id_d = nc.dram_tensor("ident", [H, H], F32R, kind="ExternalInput")
    ones_d = nc.dram_tensor("ones", [H, P], F32R, kind="ExternalInput")
    out_d = nc.dram_tensor("out", [B_LOC, L, C], F32, kind="ExternalOutput")
    with tile.TileContext(nc) as tc:
        _emit(tc, x_d, wT_d, bq_d, bkv_d, id_d, ones_d, out_d)
    nc.compile()
    return nc


_nc_cache = None


def _get_nc():
    global _nc_cache
    if _nc_cache is None:
        _nc_cache = build()
    return _nc_cache


def make_in_maps(x, Wq, bq, Wkv, bkv):
    x = np.ascontiguousarray(np.asarray(x, dtype=np.float32))
    wT = np.ascontiguousarray(
        np.concatenate(
            [np.asarray(Wq, np.float32), np.asarray(Wkv, np.float32)], axis=0
        ).T
    )
    bq = np.ascontiguousarray(np.asarray(bq, np.float32))
    bkv = np.ascontiguousarray(np.asarray(bkv, np.float32))
    ident = np.eye(H, dtype=np.float32)
    ones = np.full((H, P), 1.0 / H, dtype=np.float32)
    return [
        {
            "x": np.ascontiguousarray(x[i * B_LOC : (i + 1) * B_LOC]),
            "wT": wT,
            "bq": bq,
            "bkv": bkv,
            "ident": ident,
            "ones": ones,
        }
        for i in range(N_CORES)
    ]


def run(inputs, **spmd_kwargs):
    """Run on hardware; returns (full_output, BassKernelResults)."""
    nc = _get_nc()
    in_maps = make_in_maps(**inputs)
    res = run_bass_kernel_spmd(nc, in_maps, list(range(N_CORES)), **spmd_kwargs)
    out = np.concatenate([r["out"] for r in res.results], axis=0)
    return out, res


def kernel(**inputs) -> np.ndarray:
    out, _ = run(inputs)
    return out


# revision 6
# speedup vs baseline: 1.1436x; 1.1294x over previous
"""Trainium2 Bass kernel for nn_ChannelSelfAttention.

Reference computation (per batch sample b):
    xt   = x[b].T                          # [C, L]
    q    = xt @ Wq.T + bq                  # [C, H]
    kv   = xt @ Wkv.T + bkv                # [C, 2H] -> k, v
    attn = (q * H**-0.5) @ k.T             # [C, C]  (no softmax)
    y    = attn @ v                        # [C, H]
    g    = mean(y, axis=-1)                # [C]
    out[b] = x[b] * g[None, :]             # [L, C]

Sharding: data-parallel over B across 8 cores (4 samples per core);
weights replicated. Per-core HBM traffic: 16 MiB x read + 16 MiB out
write + 3 MiB weights = 36.75 MB -> ~103 us at the 358 GB/s per-core
HBM roofline.

Pipeline design (v2):
  - x tiles come from a bufs=4 pool, one buffer per sample, so all four
    x loads issue back-to-back on the sync ring with no buffer-reuse
    waits (the v1 bufs=2 layout head-of-line blocked the sync ring for
    ~24 us per sample waiting on the gate DVE op to free a buffer).
  - The gate multiply runs IN PLACE over the x tile (out = x * g), so
    no separate output tile is needed; the store DMAs straight from the
    x tile. Gate+store run at quarter-sample granularity to shorten the
    critical tail after the last x load.
  - l = p*32 + n*4 + j (partition outermost), so each partition's slice
    of x / out is one contiguous 32 KiB run of DRAM (16 KiB per
    half-load descriptor, 8 KiB per quarter-store descriptor).
  - PSUM->SBUF copies run on DVE (vector), not ACT, so the ACT ring
    only carries weight loads + output stores and a store blocked on a
    gate cannot delay the attention-chain copies.

On-device layout notes (per sample):
  - x in SBUF as [p=128, n=8, (j=4)*(c=256)].
  - qkv^T computed by accumulating W_all @ x[b] over the 32 (n,j)
    chunks (lhsT = W_all^T chunk, rhs = x chunk), giving q^T [64, 256]
    and kv^T [128, 256] in PSUM with the contraction over partitions.
  - attn^T[d, c] computed directly (lhsT = k^T d-chunk, rhs = q^T) to
    avoid transposing attn for the y matmul.
  - v^T [64, 256] is PE-transposed to v [256, 64] (two 64x128 tiles) so
    y^T = sum_d v[d, h] attn^T[d, c] accumulates naturally.
  - mean over H folded into a broadcast matmul: lhsT = (1/H) ones
    [64, 128], rhs = y^T -> g broadcast to all 128 partitions.
  - gate: DVE tensor_tensor multiply, g broadcast along the n,j axes
    via a stride-0 access pattern, written back over the x tile.
"""

import numpy as np

import concourse.bass as bass
import concourse.mybir as mybir
import concourse.tile as tile
from concourse import bacc
from concourse.bass_utils import run_bass_kernel_spmd

B, L, C, H = 32, 4096, 256, 64
N_CORES = 8
B_LOC = B // N_CORES          # samples per core
P = 128                       # SBUF partitions
JC = 4                        # L-rows per partition per chunk
NCH = L // (P * JC)           # l-chunks per sample (8)
DCH = C // P                  # d-chunks (2)
NQ = NCH // 4                 # n-chunks per gate/store quarter (2)
F32 = mybir.dt.float32
F32R = mybir.dt.float32r
SCALE = float(H) ** -0.5


def _emit(
    tc: "tile.TileContext", x_d, wT_d, bq_d, bkv_d, id_d, ones_d, out_d
) -> None:
    nc = tc.nc
    with (
        tc.tile_pool(name="singles", bufs=1) as singles,
        tc.tile_pool(name="xio", bufs=4) as xio,
        tc.tile_pool(name="oq", bufs=3) as oqp,
        tc.tile_pool(name="small", bufs=2) as small,
        tc.tile_pool(name="psum2", bufs=2, space="PSUM") as psum2,
        tc.tile_pool(name="psum1", bufs=1, space="PSUM") as psum1,
    ):
        # ---- one-time loads / constants (scalar HWDGE ring, so they
        # overlap the x loads on the sync ring) ----
        # W_all^T as [p, n, j, 3H]: l = p*32 + n*4 + j, so each
        # partition's weight slice is 24 KiB of contiguous DRAM. One
        # DMA so the AP normalizer merges (n j h) into a single 24 KiB
        # descriptor per partition — a split load stays at 3 KiB
        # descriptors and loses the packet round-robin against the
        # 16 KiB x-load descriptors (weights then take ~20 us to land).
        wT_sb = singles.tile([P, NCH, JC, 3 * H], F32R)      # 3 MiB
        wT_src = wT_d[:].rearrange("(p n j) h -> p n j h", n=NCH, j=JC)
        nc.scalar.dma_start(out=wT_sb, in_=wT_src)
        bq_sb = singles.tile([H, 1], F32)
        nc.scalar.dma_start(out=bq_sb, in_=bq_d[:].rearrange("(h o) -> h o", o=1))
        bkv_sb = singles.tile([2 * H, 1], F32)
        nc.scalar.dma_start(
            out=bkv_sb, in_=bkv_d[:].rearrange("(h o) -> h o", o=1)
        )
        # 64x64 identity living at partitions 64:128 so the v^T transpose
        # (lhsT at base partition 64) has a base-aligned rhs.
        id_tile = singles.tile([P, H], F32R)
        nc.scalar.dma_start(out=id_tile[H:P, :], in_=id_d[:])
        ident_hi = id_tile[H:P, :]
        ones_h = singles.tile([H, P], F32R)                  # filled with 1/H
        nc.scalar.dma_start(out=ones_h, in_=ones_d[:])

        HALF = NCH // 2
        for b in range(B_LOC):
            # ---- load x[b] into SBUF: [128, 8, 4*256] ----
            # l = p*32 + n*4 + j: per (partition, half) the DRAM source
            # is one 16 KiB contiguous run. Two half-loads so the qkv
            # matmuls start on half 0 while half 1 is in flight.
            x_sb = xio.tile([P, NCH, JC * C], F32R, tag="x")
            x_src = x_d[b].rearrange("(p n j) c -> p n (j c)", n=NCH, j=JC)
            for hh in range(2):
                sl = slice(hh * HALF, (hh + 1) * HALF)
                nc.sync.dma_start(out=x_sb[:, sl, :], in_=x_src[:, sl, :])

            # ---- qkv^T = W_all @ x[b]: accumulate over 8 chunks x 4 j ----
            psum_q = psum2.tile([H, C], F32, tag="q")
            psum_kv = psum2.tile([2 * H, C], F32, tag="kv")
            for n in range(NCH):
                for j in range(JC):
                    nc.tensor.matmul(
                        psum_q,
                        lhsT=wT_sb[:, n, j, 0:H],
                        rhs=x_sb[:, n, j * C : (j + 1) * C],
                        start=(n == 0 and j == 0),
                        stop=(n == NCH - 1 and j == JC - 1),
                    )
            for n in range(NCH):
                for j in range(JC):
                    nc.tensor.matmul(
                        psum_kv,
                        lhsT=wT_sb[:, n, j, H : 3 * H],
                        rhs=x_sb[:, n, j * C : (j + 1) * C],
                        start=(n == 0 and j == 0),
                        stop=(n == NCH - 1 and j == JC - 1),
                    )

            # q^T scaled+biased; kv^T biased (per-partition bias)
            q_sb = small.tile([H, C], F32R, tag="q_sb")
            nc.vector.tensor_scalar(
                out=q_sb,
                in0=psum_q,
                scalar1=bq_sb,
                scalar2=SCALE,
                op0=mybir.AluOpType.add,
                op1=mybir.AluOpType.mult,
            )
            kv_sb = small.tile([2 * H, C], F32R, tag="kv_sb")
            nc.vector.tensor_scalar(
                out=kv_sb,
                in0=psum_kv,
                scalar1=bkv_sb,
                scalar2=None,
                op0=mybir.AluOpType.add,
            )
            kT = kv_sb[0:H, :]                    # [64, 256]
            vT = kv_sb[H : 2 * H, :]              # [64, 256]

            # ---- v natural [d, h]: PE-transpose the two vT halves ----
            psum_vt = psum1.tile([P, P], F32R, tag="vt")
            for d in range(DCH):
                nc.tensor.transpose(
                    psum_vt[:, d * H : (d + 1) * H],
                    vT[:, d * P : (d + 1) * P],
                    ident_hi,
                )
            v_sb = small.tile([P, P], F32R, tag="v_sb")
            nc.vector.tensor_copy(v_sb, psum_vt)

            # ---- attn^T[d, c] = sum_h k^T[h, d] * q^T[h, c] ----
            psum_at = psum1.tile([P, DCH * C], F32, tag="at")
            for d in range(DCH):
                nc.tensor.matmul(
                    psum_at[:, d * C : (d + 1) * C],
                    lhsT=kT[:, d * P : (d + 1) * P],
                    rhs=q_sb[:],
                )
            at_sb = small.tile([P, DCH * C], F32R, tag="at_sb")
            nc.vector.tensor_copy(at_sb, psum_at)

            # ---- y^T[h, c] = sum_d v[d, h] * attn^T[d, c] ----
            psum_yt = psum1.tile([H, C], F32, tag="yt")
            for d in range(DCH):
                nc.tensor.matmul(
                    psum_yt,
                    lhsT=v_sb[:, d * H : (d + 1) * H],
                    rhs=at_sb[:, d * C : (d + 1) * C],
                    start=(d == 0),
                    stop=(d == DCH - 1),
                )
            yt_sb = small.tile([H, C], F32R, tag="yt_sb")
            nc.vector.tensor_copy(yt_sb, psum_yt)

            # ---- g = mean_h y^T, broadcast to all 128 partitions ----
            psum_g = psum1.tile([P, C], F32, tag="g")
            nc.tensor.matmul(psum_g, lhsT=ones_h[:], rhs=yt_sb[:])
            g_sb = small.tile([P, C], F32, tag="g_sb")
            nc.vector.tensor_copy(g_sb, psum_g)

            # ---- gate (out = x * g, g broadcast along n,j via stride 0)
            # and store, at quarter granularity so each store overlaps
            # the next quarter's gate. The staging tile is separate from
            # x_sb (an in-place gate trips the FP32r matmul verifier). ----
            out_dst = out_d[b].rearrange("(p n j) c -> p n (j c)", n=NCH, j=JC)
            g_bc = bass.AP(
                tensor=g_sb.tensor,
                offset=g_sb.offset,
                ap=[list(g_sb.ap[0]), [0, NQ], [0, JC], list(g_sb.ap[1])],
            )
            for qq in range(4):
                sl = slice(qq * NQ, (qq + 1) * NQ)
                oq = oqp.tile([P, NQ, JC * C], F32, tag="oq")
                nc.vector.tensor_tensor(
                    out=oq.rearrange("p n (j c) -> p n j c", j=JC),
                    in0=x_sb[:, sl, :]
                    .bitcast(F32)
                    .rearrange("p n (j c) -> p n j c", j=JC),
                    in1=g_bc,
                    op=mybir.AluOpType.mult,
                )
                nc.scalar.dma_start(out=out_dst[:, sl, :], in_=oq)


def build():
    nc = bacc.Bacc(
        "TRN2", target_bir_lowering=False, debug=False, num_devices=N_CORES
    )
    x_d = nc.dram_tensor("x", [B_LOC, L, C], F32R, kind="ExternalInput")
    wT_d = nc.dram_tensor("wT", [L, 3 * H], F32R, kind="ExternalInput")
    bq_d = nc.dram_tensor("bq", [H], F32, kind="ExternalInput")
    bkv_d = nc.dram_tensor("bkv", [2 * H], F32, kind="ExternalInput")
    id_d = nc.dram_tensor("ident", [H, H], F32R, kind="ExternalInput")
    ones_d = nc.dram_tensor("ones", [H, P], F32R, kind="ExternalInput")
    out_d = nc.dram_tensor("out", [B_LOC, L, C], F32, kind="ExternalOutput")
    with tile.TileContext(nc) as tc:
        _emit(tc, x_d, wT_d, bq_d, bkv_d, id_d, ones_d, out_d)
    nc.compile()
    return nc


_nc_cache = None


def _get_nc():
    global _nc_cache
    if _nc_cache is None:
        _nc_cache = build()
    return _nc_cache


def make_in_maps(x, Wq, bq, Wkv, bkv):
    x = np.ascontiguousarray(np.asarray(x, dtype=np.float32))
    wT = np.ascontiguousarray(
        np.concatenate(
            [np.asarray(Wq, np.float32), np.asarray(Wkv, np.float32)], axis=0
        ).T
    )
    bq = np.ascontiguousarray(np.asarray(bq, np.float32))
    bkv = np.ascontiguousarray(np.asarray(bkv, np.float32))
    ident = np.eye(H, dtype=np.float32)
    ones = np.full((H, P), 1.0 / H, dtype=np.float32)
    return [
        {
            "x": np.ascontiguousarray(x[i * B_LOC : (i + 1) * B_LOC]),
            "wT": wT,
            "bq": bq,
            "bkv": bkv,
            "ident": ident,
            "ones": ones,
        }
        for i in range(N_CORES)
    ]


def run(inputs, **spmd_kwargs):
    """Run on hardware; returns (full_output, BassKernelResults)."""
    nc = _get_nc()
    in_maps = make_in_maps(**inputs)
    res = run_bass_kernel_spmd(nc, in_maps, list(range(N_CORES)), **spmd_kwargs)
    out = np.concatenate([r["out"] for r in res.results], axis=0)
    return out, res


def kernel(**inputs) -> np.ndarray:
    out, _ = run(inputs)
    return out


# revision 8
# speedup vs baseline: 1.1452x; 1.0014x over previous
"""Trainium2 Bass kernel for nn_ChannelSelfAttention.

Reference computation (per batch sample b):
    xt   = x[b].T                          # [C, L]
    q    = xt @ Wq.T + bq                  # [C, H]
    kv   = xt @ Wkv.T + bkv                # [C, 2H] -> k, v
    attn = (q * H**-0.5) @ k.T             # [C, C]  (no softmax)
    y    = attn @ v                        # [C, H]
    g    = mean(y, axis=-1)                # [C]
    out[b] = x[b] * g[None, :]             # [L, C]

Sharding: data-parallel over B across 8 cores (4 samples per core);
weights replicated. Per-core HBM traffic: 16 MiB x read + 16 MiB out
write + 3 MiB weights = 36.75 MB -> ~103 us at the 358 GB/s per-core
HBM roofline.

Pipeline design (v2):
  - x tiles come from a bufs=4 pool, one buffer per sample, so all four
    x loads issue back-to-back on the sync ring with no buffer-reuse
    waits (the v1 bufs=2 layout head-of-line blocked the sync ring for
    ~24 us per sample waiting on the gate DVE op to free a buffer).
  - The gate multiply runs IN PLACE over the x tile (out = x * g), so
    no separate output tile is needed; the store DMAs straight from the
    x tile. Gate+store run at quarter-sample granularity to shorten the
    critical tail after the last x load.
  - l = p*32 + n*4 + j (partition outermost), so each partition's slice
    of x / out is one contiguous 32 KiB run of DRAM (16 KiB per
    half-load descriptor, 8 KiB per quarter-store descriptor).
  - PSUM->SBUF copies run on DVE (vector), not ACT, so the ACT ring
    only carries weight loads + output stores and a store blocked on a
    gate cannot delay the attention-chain copies.

On-device layout notes (per sample):
  - x in SBUF as [p=128, n=8, (j=4)*(c=256)].
  - qkv^T computed by accumulating W_all @ x[b] over the 32 (n,j)
    chunks (lhsT = W_all^T chunk, rhs = x chunk), giving q^T [64, 256]
    and kv^T [128, 256] in PSUM with the contraction over partitions.
  - attn^T[d, c] computed directly (lhsT = k^T d-chunk, rhs = q^T) to
    avoid transposing attn for the y matmul.
  - v^T [64, 256] is PE-transposed to v [256, 64] (two 64x128 tiles) so
    y^T = sum_d v[d, h] attn^T[d, c] accumulates naturally.
  - mean over H folded into a broadcast matmul: lhsT = (1/H) ones
    [64, 128], rhs = y^T -> g broadcast to all 128 partitions.
  - gate: DVE tensor_tensor multiply, g broadcast along the n,j axes
    via a stride-0 access pattern, written back over the x tile.
"""

import numpy as np

import concourse.bass as bass
import concourse.mybir as mybir
import concourse.tile as tile
from concourse import bacc
from concourse.bass_utils import run_bass_kernel_spmd

B, L, C, H = 32, 4096, 256, 64
N_CORES = 8
B_LOC = B // N_CORES          # samples per core
P = 128                       # SBUF partitions
JC = 4                        # L-rows per partition per chunk
NCH = L // (P * JC)           # l-chunks per sample (8)
DCH = C // P                  # d-chunks (2)
NQ = NCH // 4                 # n-chunks per gate/store quarter (2)
F32 = mybir.dt.float32
F32R = mybir.dt.float32r
SCALE = float(H) ** -0.5


def _emit(
    tc: "tile.TileContext", x_d, wT_d, bq_d, bkv_d, id_d, ones_d, out_d
) -> None:
    nc = tc.nc
    with (
        tc.tile_pool(name="singles", bufs=1) as singles,
        tc.tile_pool(name="xio", bufs=4) as xio,
        tc.tile_pool(name="oq", bufs=3) as oqp,
        tc.tile_pool(name="small", bufs=2) as small,
        tc.tile_pool(name="psum2", bufs=2, space="PSUM") as psum2,
        tc.tile_pool(name="psum1", bufs=1, space="PSUM") as psum1,
    ):
        # ---- one-time loads / constants (scalar HWDGE ring, so they
        # overlap the x loads on the sync ring) ----
        # W_all^T as [p, n, j, 3H]: l = p*32 + n*4 + j, so each
        # partition's weight slice is 24 KiB of contiguous DRAM. One
        # DMA so the AP normalizer merges (n j h) into a single 24 KiB
        # descriptor per partition — a split load stays at 3 KiB
        # descriptors and loses the packet round-robin against the
        # 16 KiB x-load descriptors (weights then take ~20 us to land).
        wT_sb = singles.tile([P, NCH, JC, 3 * H], F32R)      # 3 MiB
        wT_src = wT_d[:].rearrange("(p n j) h -> p n j h", n=NCH, j=JC)
        nc.scalar.dma_start(out=wT_sb[:, 0 : NCH // 2], in_=wT_src[:, 0 : NCH // 2])
        nc.scalar.dma_start(out=wT_sb[:, NCH // 2 :], in_=wT_src[:, NCH // 2 :])
        bq_sb = singles.tile([H, 1], F32)
        nc.scalar.dma_start(out=bq_sb, in_=bq_d[:].rearrange("(h o) -> h o", o=1))
        bkv_sb = singles.tile([2 * H, 1], F32)
        nc.scalar.dma_start(
            out=bkv_sb, in_=bkv_d[:].rearrange("(h o) -> h o", o=1)
        )
        # 64x64 identity living at partitions 64:128 so the v^T transpose
        # (lhsT at base partition 64) has a base-aligned rhs.
        id_tile = singles.tile([P, H], F32R)
        nc.scalar.dma_start(out=id_tile[H:P, :], in_=id_d[:])
        ident_hi = id_tile[H:P, :]
        ones_h = singles.tile([H, P], F32R)                  # filled with 1/H
        nc.scalar.dma_start(out=ones_h, in_=ones_d[:])

        HALF = NCH // 2
        for b in range(B_LOC):
            # ---- load x[b] into SBUF: [128, 8, 4*256] ----
            # l = p*32 + n*4 + j: per (partition, half) the DRAM source
            # is one 16 KiB contiguous run. Two half-loads so the qkv
            # matmuls start on half 0 while half 1 is in flight.
            x_sb = xio.tile([P, NCH, JC * C], F32R, tag="x")
            x_src = x_d[b].rearrange("(p n j) c -> p n (j c)", n=NCH, j=JC)
            for hh in range(2):
                sl = slice(hh * HALF, (hh + 1) * HALF)
                nc.sync.dma_start(out=x_sb[:, sl, :], in_=x_src[:, sl, :])

            # ---- qkv^T = W_all @ x[b]: accumulate over 8 chunks x 4 j ----
            psum_q = psum2.tile([H, C], F32, tag="q")
            psum_kv = psum2.tile([2 * H, C], F32, tag="kv")
            for n in range(NCH):
                for j in range(JC):
                    nc.tensor.matmul(
                        psum_q,
                        lhsT=wT_sb[:, n, j, 0:H],
                        rhs=x_sb[:, n, j * C : (j + 1) * C],
                        start=(n == 0 and j == 0),
                        stop=(n == NCH - 1 and j == JC - 1),
                    )
            for n in range(NCH):
                for j in range(JC):
                    nc.tensor.matmul(
                        psum_kv,
                        lhsT=wT_sb[:, n, j, H : 3 * H],
                        rhs=x_sb[:, n, j * C : (j + 1) * C],
                        start=(n == 0 and j == 0),
                        stop=(n == NCH - 1 and j == JC - 1),
                    )

            # q^T scaled+biased; kv^T biased (per-partition bias)
            q_sb = small.tile([H, C], F32R, tag="q_sb")
            nc.vector.tensor_scalar(
                out=q_sb,
                in0=psum_q,
                scalar1=bq_sb,
                scalar2=SCALE,
                op0=mybir.AluOpType.add,
                op1=mybir.AluOpType.mult,
            )
            kv_sb = small.tile([2 * H, C], F32R, tag="kv_sb")
            nc.vector.tensor_scalar(
                out=kv_sb,
                in0=psum_kv,
                scalar1=bkv_sb,
                scalar2=None,
                op0=mybir.AluOpType.add,
            )
            kT = kv_sb[0:H, :]                    # [64, 256]
            vT = kv_sb[H : 2 * H, :]              # [64, 256]

            # ---- v natural [d, h]: PE-transpose the two vT halves ----
            psum_vt = psum1.tile([P, P], F32R, tag="vt")
            for d in range(DCH):
                nc.tensor.transpose(
                    psum_vt[:, d * H : (d + 1) * H],
                    vT[:, d * P : (d + 1) * P],
                    ident_hi,
                )
            v_sb = small.tile([P, P], F32R, tag="v_sb")
            nc.vector.tensor_copy(v_sb, psum_vt)

            # ---- attn^T[d, c] = sum_h k^T[h, d] * q^T[h, c] ----
            psum_at = psum1.tile([P, DCH * C], F32, tag="at")
            for d in range(DCH):
                nc.tensor.matmul(
                    psum_at[:, d * C : (d + 1) * C],
                    lhsT=kT[:, d * P : (d + 1) * P],
                    rhs=q_sb[:],
                )
            at_sb = small.tile([P, DCH * C], F32R, tag="at_sb")
            nc.vector.tensor_copy(at_sb, psum_at)

            # ---- y^T[h, c] = sum_d v[d, h] * attn^T[d, c] ----
            psum_yt = psum1.tile([H, C], F32, tag="yt")
            for d in range(DCH):
                nc.tensor.matmul(
                    psum_yt,
                    lhsT=v_sb[:, d * H : (d + 1) * H],
                    rhs=at_sb[:, d * C : (d + 1) * C],
                    start=(d == 0),
                    stop=(d == DCH - 1),
                )
            yt_sb = small.tile([H, C], F32R, tag="yt_sb")
            nc.vector.tensor_copy(yt_sb, psum_yt)

            # ---- g = mean_h y^T, broadcast to all 128 partitions ----
            psum_g = psum1.tile([P, C], F32, tag="g")
            nc.tensor.matmul(psum_g, lhsT=ones_h[:], rhs=yt_sb[:])
            g_sb = small.tile([P, C], F32, tag="g_sb")
            nc.vector.tensor_copy(g_sb, psum_g)

            # ---- gate (out = x * g, g broadcast along n,j via stride 0)
            # and store, at quarter granularity so each store overlaps
            # the next quarter's gate. The staging tile is separate from
            # x_sb (an in-place gate trips the FP32r matmul verifier). ----
            out_dst = out_d[b].rearrange("(p n j) c -> p n (j c)", n=NCH, j=JC)
            g_bc = bass.AP(
                tensor=g_sb.tensor,
                offset=g_sb.offset,
                ap=[list(g_sb.ap[0]), [0, NQ], [0, JC], list(g_sb.ap[1])],
            )
            for qq in range(4):
                sl = slice(qq * NQ, (qq + 1) * NQ)
                oq = oqp.tile([P, NQ, JC * C], F32, tag="oq")
                # Alternate the gate multiply between DVE and GpSimd —
                # the serial ~8.9 us/sample DVE gate backlog otherwise
                # stalls the store stream at the tail of the run.
                eng = nc.vector if qq % 2 == 0 else nc.gpsimd
                eng.tensor_tensor(
                    out=oq.rearrange("p n (j c) -> p n j c", j=JC),
                    in0=x_sb[:, sl, :]
                    .bitcast(F32)
                    .rearrange("p n (j c) -> p n j c", j=JC),
                    in1=g_bc,
                    op=mybir.AluOpType.mult,
                )
                nc.scalar.dma_start(out=out_dst[:, sl, :], in_=oq)


def build():
    nc = bacc.Bacc(
        "TRN2", target_bir_lowering=False, debug=False, num_devices=N_CORES
    )
    x_d = nc.dram_tensor("x", [B_LOC, L, C], F32R, kind="ExternalInput")
    wT_d = nc.dram_tensor("wT", [L, 3 * H], F32R, kind="ExternalInput")
    bq_d = nc.dram_tensor("bq", [H], F32, kind="ExternalInput")
    bkv_d = nc.dram_tensor("bkv", [2 * H], F32, kind="ExternalInput")
    id_d = nc.dram_tensor("ident", [H, H], F32R, kind="ExternalInput")
    ones_d = nc.dram_tensor("ones", [H, P], F32R, kind="ExternalInput")
    out_d = nc.dram_tensor("out", [B_LOC, L, C], F32, kind="ExternalOutput")
    with tile.TileContext(nc) as tc:
        _emit(tc, x_d, wT_d, bq_d, bkv_d, id_d, ones_d, out_d)
    nc.compile()
    return nc


_nc_cache = None


def _get_nc():
    global _nc_cache
    if _nc_cache is None:
        _nc_cache = build()
    return _nc_cache


def make_in_maps(x, Wq, bq, Wkv, bkv):
    x = np.ascontiguousarray(np.asarray(x, dtype=np.float32))
    wT = np.ascontiguousarray(
        np.concatenate(
            [np.asarray(Wq, np.float32), np.asarray(Wkv, np.float32)], axis=0
        ).T
    )
    bq = np.ascontiguousarray(np.asarray(bq, np.float32))
    bkv = np.ascontiguousarray(np.asarray(bkv, np.float32))
    ident = np.eye(H, dtype=np.float32)
    ones = np.full((H, P), 1.0 / H, dtype=np.float32)
    return [
        {
            "x": np.ascontiguousarray(x[i * B_LOC : (i + 1) * B_LOC]),
            "wT": wT,
            "bq": bq,
            "bkv": bkv,
            "ident": ident,
            "ones": ones,
        }
        for i in range(N_CORES)
    ]


def run(inputs, **spmd_kwargs):
    """Run on hardware; returns (full_output, BassKernelResults)."""
    nc = _get_nc()
    in_maps = make_in_maps(**inputs)
    res = run_bass_kernel_spmd(nc, in_maps, list(range(N_CORES)), **spmd_kwargs)
    out = np.concatenate([r["out"] for r in res.results], axis=0)
    return out, res


def kernel(**inputs) -> np.ndarray:
    out, _ = run(inputs)
    return out


# revision 9
# speedup vs baseline: 1.1640x; 1.0164x over previous
"""Trainium2 Bass kernel for nn_ChannelSelfAttention.

Reference computation (per batch sample b):
    xt   = x[b].T                          # [C, L]
    q    = xt @ Wq.T + bq                  # [C, H]
    kv   = xt @ Wkv.T + bkv                # [C, 2H] -> k, v
    attn = (q * H**-0.5) @ k.T             # [C, C]  (no softmax)
    y    = attn @ v                        # [C, H]
    g    = mean(y, axis=-1)                # [C]
    out[b] = x[b] * g[None, :]             # [L, C]

Sharding: data-parallel over B across 8 cores (4 samples per core);
weights replicated. Per-core HBM traffic: 16 MiB x read + 16 MiB out
write + 3 MiB weights = 36.75 MB. The two HWDGE rings together sustain
~430 GB/s, so the DMA floor is ~86 us; everything else must hide under
it.

Pipeline design (v6):
  - x tiles from a bufs=4 pool, one buffer per sample: all four x loads
    issue back-to-back on the sync ring with no buffer-reuse waits.
  - l = p*32 + n*4 + j (partition outermost): each partition's slice of
    x / out is contiguous DRAM (16 KiB per half-load descriptor, 8 KiB
    per quarter-store descriptor). Weights load in two DMAs (12 KiB
    descriptors) so sample 0's qkv can start after the first half; a
    finer split would shrink descriptors below the packet round-robin
    sweet spot and starve the weight stream behind the x stream.
  - Software-pipelined PE stream: emit qkv[b], then the attention chain
    + gate + store of sample b-1. The PE never sits behind a
    cross-engine round-trip before starting the next sample's qkv, and
    after the last x load only one sample's chain remains.
  - PSUM->SBUF copies on ACT (scalar), gates on DVE only. A GpSimd
    gate assist was measured net-negative: GpSimd tensor_tensor is
    2-3x slower and its SBUF traffic degrades concurrent DVE ops ~2x.
  - Gate + store at quarter-sample granularity so the store stream is
    fed continuously and the last quarter's store tail is short.

On-device layout notes (per sample):
  - x in SBUF as [p=128, n=8, (j=4)*(c=256)].
  - qkv^T computed by accumulating W_all @ x[b] over the 32 (n,j)
    chunks (lhsT = W_all^T chunk, rhs = x chunk), giving q^T [64, 256]
    and kv^T [128, 256] in PSUM with the contraction over partitions.
  - attn^T[d, c] computed directly (lhsT = k^T d-chunk, rhs = q^T) to
    avoid transposing attn for the y matmul.
  - v^T [64, 256] is PE-transposed to v [256, 64] (two 64x128 tiles) so
    y^T = sum_d v[d, h] attn^T[d, c] accumulates naturally.
  - mean over H folded into a broadcast matmul: lhsT = (1/H) ones
    [64, 128], rhs = y^T -> g broadcast to all 128 partitions.
  - gate: DVE tensor_tensor multiply, g broadcast along the n,j axes
    via a stride-0 access pattern, into a small staging tile (an
    in-place gate over x_sb trips the FP32r matmul verifier).
"""

import numpy as np

import concourse.bass as bass
import concourse.mybir as mybir
import concourse.tile as tile
from concourse import bacc
from concourse.bass_utils import run_bass_kernel_spmd

B, L, C, H = 32, 4096, 256, 64
N_CORES = 8
B_LOC = B // N_CORES          # samples per core
P = 128                       # SBUF partitions
JC = 4                        # L-rows per partition per chunk
NCH = L // (P * JC)           # l-chunks per sample (8)
DCH = C // P                  # d-chunks (2)
NQ = NCH // 4                 # n-chunks per gate/store quarter (2)
F32 = mybir.dt.float32
F32R = mybir.dt.float32r
SCALE = float(H) ** -0.5


def _emit(
    tc: "tile.TileContext", x_d, wT_d, bq_d, bkv_d, id_d, ones_d, out_d
) -> None:
    nc = tc.nc
    with (
        tc.tile_pool(name="singles", bufs=1) as singles,
        tc.tile_pool(name="xio", bufs=4) as xio,
        tc.tile_pool(name="oq", bufs=3) as oqp,
        tc.tile_pool(name="small", bufs=2) as small,
        tc.tile_pool(name="psum2", bufs=2, space="PSUM") as psum2,
        tc.tile_pool(name="psum1", bufs=1, space="PSUM") as psum1,
    ):
        # ---- one-time loads / constants (scalar HWDGE ring, so they
        # overlap the x loads on the sync ring) ----
        wT_sb = singles.tile([P, NCH, JC, 3 * H], F32R)      # 3 MiB
        wT_src = wT_d[:].rearrange("(p n j) h -> p n j h", n=NCH, j=JC)
        nc.scalar.dma_start(out=wT_sb[:, 0 : NCH // 2], in_=wT_src[:, 0 : NCH // 2])
        nc.scalar.dma_start(out=wT_sb[:, NCH // 2 :], in_=wT_src[:, NCH // 2 :])
        bq_sb = singles.tile([H, 1], F32)
        nc.scalar.dma_start(out=bq_sb, in_=bq_d[:].rearrange("(h o) -> h o", o=1))
        bkv_sb = singles.tile([2 * H, 1], F32)
        nc.scalar.dma_start(
            out=bkv_sb, in_=bkv_d[:].rearrange("(h o) -> h o", o=1)
        )
        # 64x64 identity living at partitions 64:128 so the v^T transpose
        # (lhsT at base partition 64) has a base-aligned rhs.
        id_tile = singles.tile([P, H], F32R)
        nc.scalar.dma_start(out=id_tile[H:P, :], in_=id_d[:])
        ident_hi = id_tile[H:P, :]
        ones_h = singles.tile([H, P], F32R)                  # filled with 1/H
        nc.scalar.dma_start(out=ones_h, in_=ones_d[:])

        HALF = NCH // 2

        def load_qkv(b):
            """Load x[b]; project to q^T/kv^T in PSUM."""
            x_sb = xio.tile([P, NCH, JC * C], F32R, tag="x")
            x_src = x_d[b].rearrange("(p n j) c -> p n (j c)", n=NCH, j=JC)
            for hh in range(2):
                sl = slice(hh * HALF, (hh + 1) * HALF)
                nc.sync.dma_start(out=x_sb[:, sl, :], in_=x_src[:, sl, :])

            psum_q = psum2.tile([H, C], F32, tag="q")
            psum_kv = psum2.tile([2 * H, C], F32, tag="kv")
            for n in range(NCH):
                for j in range(JC):
                    nc.tensor.matmul(
                        psum_q,
                        lhsT=wT_sb[:, n, j, 0:H],
                        rhs=x_sb[:, n, j * C : (j + 1) * C],
                        start=(n == 0 and j == 0),
                        stop=(n == NCH - 1 and j == JC - 1),
                    )
            for n in range(NCH):
                for j in range(JC):
                    nc.tensor.matmul(
                        psum_kv,
                        lhsT=wT_sb[:, n, j, H : 3 * H],
                        rhs=x_sb[:, n, j * C : (j + 1) * C],
                        start=(n == 0 and j == 0),
                        stop=(n == NCH - 1 and j == JC - 1),
                    )
            return x_sb, psum_q, psum_kv

        def attn_gate_store(b, x_sb, psum_q, psum_kv):
            """Finish sample b: attention chain, gate, store."""
            # q^T scaled+biased; kv^T biased (per-partition bias)
            q_sb = small.tile([H, C], F32R, tag="q_sb")
            nc.vector.tensor_scalar(
                out=q_sb,
                in0=psum_q,
                scalar1=bq_sb,
                scalar2=SCALE,
                op0=mybir.AluOpType.add,
                op1=mybir.AluOpType.mult,
            )
            kv_sb = small.tile([2 * H, C], F32R, tag="kv_sb")
            nc.vector.tensor_scalar(
                out=kv_sb,
                in0=psum_kv,
                scalar1=bkv_sb,
                scalar2=None,
                op0=mybir.AluOpType.add,
            )
            kT = kv_sb[0:H, :]                    # [64, 256]
            vT = kv_sb[H : 2 * H, :]              # [64, 256]

            # ---- v natural [d, h]: PE-transpose the two vT halves ----
            psum_vt = psum1.tile([P, P], F32R, tag="vt")
            for d in range(DCH):
                nc.tensor.transpose(
                    psum_vt[:, d * H : (d + 1) * H],
                    vT[:, d * P : (d + 1) * P],
                    ident_hi,
                )
            v_sb = small.tile([P, P], F32R, tag="v_sb")
            nc.scalar.copy(v_sb, psum_vt)

            # ---- attn^T[d, c] = sum_h k^T[h, d] * q^T[h, c] ----
            psum_at = psum1.tile([P, DCH * C], F32, tag="at")
            for d in range(DCH):
                nc.tensor.matmul(
                    psum_at[:, d * C : (d + 1) * C],
                    lhsT=kT[:, d * P : (d + 1) * P],
                    rhs=q_sb[:],
                )
            at_sb = small.tile([P, DCH * C], F32R, tag="at_sb")
            nc.scalar.copy(at_sb, psum_at)

            # ---- y^T[h, c] = sum_d v[d, h] * attn^T[d, c] ----
            psum_yt = psum1.tile([H, C], F32, tag="yt")
            for d in range(DCH):
                nc.tensor.matmul(
                    psum_yt,
                    lhsT=v_sb[:, d * H : (d + 1) * H],
                    rhs=at_sb[:, d * C : (d + 1) * C],
                    start=(d == 0),
                    stop=(d == DCH - 1),
                )
            yt_sb = small.tile([H, C], F32R, tag="yt_sb")
            nc.scalar.copy(yt_sb, psum_yt)

            # ---- g = mean_h y^T, broadcast to all 128 partitions ----
            psum_g = psum1.tile([P, C], F32, tag="g")
            nc.tensor.matmul(psum_g, lhsT=ones_h[:], rhs=yt_sb[:])
            g_sb = small.tile([P, C], F32, tag="g_sb")
            nc.scalar.copy(g_sb, psum_g)

            # ---- gate (out = x * g, g broadcast along n,j via stride 0)
            # and store, at quarter granularity ----
            out_dst = out_d[b].rearrange("(p n j) c -> p n (j c)", n=NCH, j=JC)
            g_bc = bass.AP(
                tensor=g_sb.tensor,
                offset=g_sb.offset,
                ap=[list(g_sb.ap[0]), [0, NQ], [0, JC], list(g_sb.ap[1])],
            )
            for qq in range(4):
                sl = slice(qq * NQ, (qq + 1) * NQ)
                oq = oqp.tile([P, NQ, JC * C], F32, tag="oq")
                nc.vector.tensor_tensor(
                    out=oq.rearrange("p n (j c) -> p n j c", j=JC),
                    in0=x_sb[:, sl, :]
                    .bitcast(F32)
                    .rearrange("p n (j c) -> p n j c", j=JC),
                    in1=g_bc,
                    op=mybir.AluOpType.mult,
                )
                nc.scalar.dma_start(out=out_dst[:, sl, :], in_=oq)

        # Software pipeline: qkv[b] is emitted before the attention
        # chain of b-1, so the PE queue is never head-of-line blocked
        # on a DVE/ACT round-trip before the next sample's projection.
        pend = None
        for b in range(B_LOC):
            cur = load_qkv(b)
            if pend is not None:
                attn_gate_store(b - 1, *pend)
            pend = cur
        attn_gate_store(B_LOC - 1, *pend)


def build():
    nc = bacc.Bacc(
        "TRN2", target_bir_lowering=False, debug=False, num_devices=N_CORES
    )
    x_d = nc.dram_tensor("x", [B_LOC, L, C], F32R, kind="ExternalInput")
    wT_d = nc.dram_tensor("wT", [L, 3 * H], F32R, kind="ExternalInput")
    bq_d = nc.dram_tensor("bq", [H], F32, kind="ExternalInput")
    bkv_d = nc.dram_tensor("bkv", [2 * H], F32, kind="ExternalInput")
    id_d = nc.dram_tensor("ident", [H, H], F32R, kind="ExternalInput")
    ones_d = nc.dram_tensor("ones", [H, P], F32R, kind="ExternalInput")
    out_d = nc.dram_tensor("out", [B_LOC, L, C], F32, kind="ExternalOutput")
    with tile.TileContext(nc) as tc:
        _emit(tc, x_d, wT_d, bq_d, bkv_d, id_d, ones_d, out_d)
    nc.compile()
    return nc


_nc_cache = None


def _get_nc():
    global _nc_cache
    if _nc_cache is None:
        _nc_cache = build()
    return _nc_cache


def make_in_maps(x, Wq, bq, Wkv, bkv):
    x = np.ascontiguousarray(np.asarray(x, dtype=np.float32))
    wT = np.ascontiguousarray(
        np.concatenate(
            [np.asarray(Wq, np.float32), np.asarray(Wkv, np.float32)], axis=0
        ).T
    )
    bq = np.ascontiguousarray(np.asarray(bq, np.float32))
    bkv = np.ascontiguousarray(np.asarray(bkv, np.float32))
    ident = np.eye(H, dtype=np.float32)
    ones = np.full((H, P), 1.0 / H, dtype=np.float32)
    return [
        {
            "x": np.ascontiguousarray(x[i * B_LOC : (i + 1) * B_LOC]),
            "wT": wT,
            "bq": bq,
            "bkv": bkv,
            "ident": ident,
            "ones": ones,
        }
        for i in range(N_CORES)
    ]


def run(inputs, **spmd_kwargs):
    """Run on hardware; returns (full_output, BassKernelResults)."""
    nc = _get_nc()
    in_maps = make_in_maps(**inputs)
    res = run_bass_kernel_spmd(nc, in_maps, list(range(N_CORES)), **spmd_kwargs)
    out = np.concatenate([r["out"] for r in res.results], axis=0)
    return out, res


def kernel(**inputs) -> np.ndarray:
    out, _ = run(inputs)
    return out
